# revision 2
# baseline (speedup 1.0000x reference)
"""Trainium2 Bass kernel for the few-shot knn-attention module.

Math (single AllGather design, v1): per-sample softmax mask over (C*H*W),
masked features, class prototypes from 25 shots sharded across 8 cores,
cosine-similarity scores for 75 queries, class softmax, spatial mean ->
(75, 5).  Slots: 4 shot + 10 query per core; prototype partials exchanged
with one AllGather (the only cross-core sync the timeline cost model can
represent; peer remote-DMA exchanges deadlock TimelineSim(no_exec=True)).

Optimizations vs the 41316ns baseline (timeline-sim driven, 40598ns):
  - Early DMAs batched into byte-packed uint8 tensors with bitcast views:
    pk0=[wt8|x8_0|bias+sw], pk1=[x8_1..3], xbt in 2 chunks, query fp8 in
    2x5-slot chunks.  SP dispatch queue carries {early chunks, x8qA,
    ar_in, x8qB, xbq chunks, protoAll, out}; ar_in's HWDGE/DGE prologue
    overlaps its data wait so the collective launches ~1.6us after the
    prototype reduce.
  - Shot chain at priority 0 end-to-end (conv/exp/mul/column-matmuls):
    the in-order engine sequencers never park on query-side work before
    the collective input is ready; bias/sw ride in pk0 so the eb tables
    are built before the first shot exp.
  - PSUM accumulation groups are kept contiguous: each start/stop matmul
    pair is emitted back-to-back.  Interleaving other matmuls inside an
    open accumulation group corrupts it on hardware (observed as a
    uniform ~3e-2 error; the timeline sim does not model this rule).
  - Class softmax batched across all 10 queries (v1 scheme) with the
    10/||s|| class-norm factor applied on the logits, off the
    collective->scores critical path.

Known remaining fat (documented for a future session): the collective's
15us cost-model constant (~39% of total), the 2.2us HWDGE+DGE+sem-prop
prologue on each of the protoAll-landing and out DMAs (a prepare_only
kv_writeback/trigger_dma path would prepay desc-gen, but tile's DMASW
lane-sem bookkeeping requires sem=tc.sems.swdge_block()[lane] in tick
order, and the resulting Pool-queue parks destabilized the schedule; a
dma_gather landing from internal DRAM faulted the device - do not retry
without CoreSim), and ~1.5us of cross-engine hops in the shot epilogue.
"""

import numpy as np
import ml_dtypes

import concourse.bass as bass
import concourse.mybir as mybir
import concourse.tile as tile
from concourse import bacc
from concourse.bass_utils import run_bass_kernel_spmd

# Force the act-table chooser onto the one set containing BOTH Exp and Ln
# ("natural_log_exp_and_others") so the kernel pays a single table load.
import concourse.hw_specs as _hw_specs

_ORIG_GET_ACT_TABLES = _hw_specs.get_activation_tables


def _nl_exp_only_tables(arch):
    t = _ORIG_GET_ACT_TABLES(arch)
    return {
        k: (v if k == "natural_log_exp_and_others" else set()) for k, v in t.items()
    }


bacc.get_activation_tables = _nl_exp_only_tables

N_CORES = 8
WAY = 5
SHOT = 5
C = 512
F = 256  # 16*16
KO = C // 128
NSHOT_SLOTS = 4
NQ_SLOTS = 10
NSLOTS = NSHOT_SLOTS + NQ_SLOTS
SHOTS_PER_CORE = [4, 3, 3, 3, 3, 3, 3, 3]       # sums to 25
QUERIES_PER_CORE = [9, 10, 10, 10, 9, 9, 9, 9]  # sums to 75
LN10 = float(np.log(10.0))
W_SCALE = 16.0  # fp8 weight pre-scale; exp() descales via scale=1/W_SCALE
N_WARM = 11     # dummy matmuls bridging the PE pstate ramp at startup

F32 = mybir.dt.float32
BF16 = mybir.dt.bfloat16
FP8 = mybir.dt.float8e4
U8 = mybir.dt.uint8
EXP = mybir.ActivationFunctionType.Exp
LN = mybir.ActivationFunctionType.Ln
MULT = mybir.AluOpType.mult
ADD = mybir.AluOpType.add
DR = mybir.MatmulPerfMode.DoubleRow

WT_B = 128 * 16          # wt8 bytes per partition (KO*C fp8)
XS_B = KO * F            # one x-slot fp8 bytes per partition
BSW_B = (KO + NSHOT_SLOTS * WAY) * 4


def build_nc():
    nc = bacc.Bacc(None, target_bir_lowering=False)
    pk0 = nc.dram_tensor("pk0", [128, WT_B + XS_B + BSW_B], U8, kind="ExternalInput")
    pk1 = nc.dram_tensor("pk1", [128, 3 * XS_B], U8, kind="ExternalInput")
    # transposed bf16 shot inputs: xbt[i, f-part, (h, c)] = x[i, c, h*128+f]
    xbt = nc.dram_tensor("xbt", [NSHOT_SLOTS, 128, 2 * C], BF16, kind="ExternalInput")
    x8q = nc.dram_tensor("x8q", [NQ_SLOTS, 128, KO * F], FP8, kind="ExternalInput")
    xbq = nc.dram_tensor("xbq", [NQ_SLOTS, 128, KO * F], BF16, kind="ExternalInput")
    out = nc.dram_tensor("out", [1, NQ_SLOTS * WAY], F32, kind="ExternalOutput")

    with tile.TileContext(nc) as tc:
        with (
            tc.tile_pool(name="singles", bufs=1) as singles,
            tc.tile_pool(name="xepool", bufs=4) as xepool,
            tc.tile_pool(name="xms", bufs=2) as xms_pool,
            tc.tile_pool(name="xmq", bufs=NQ_SLOTS) as xmq_pool,
            tc.tile_pool(name="xm2", bufs=3) as xm2_pool,
            tc.tile_pool(name="work", bufs=8) as work,
            tc.tile_pool(name="pconv", bufs=2, space="PSUM") as pconv,
            tc.tile_pool(name="pscratch", bufs=1, space="PSUM") as pscratch,
            tc.tile_pool(name="psmall", bufs=1, space="PSUM") as psmall,
            tc.tile_pool(name="dram", bufs=2, space="DRAM") as dram,
        ):
            # ---------------- constants ----------------
            warm_bf = singles.tile([128, 256], BF16)
            nc.vector.memset(warm_bf, 0.0)
            onesC_f32 = singles.tile([128, 128], F32)
            nc.vector.memset(onesC_f32, 1.0)
            onesC_bf = singles.tile([128, 128], BF16)
            nc.vector.memset(onesC_bf, 1.0)
            ones_col = singles.tile([128, KO, 1], BF16)
            nc.vector.memset(ones_col, 1.0)
            c_eps = singles.tile([128, 1], F32)
            nc.vector.memset(c_eps, 1e-30)
            c_ln10 = singles.tile([128, 1], F32)
            nc.vector.memset(c_ln10, LN10)

            # ---- early DMA group (SP queue, in this order) ----
            pk0_sb = singles.tile([128, WT_B + XS_B + BSW_B], U8)
            pk1_sb = singles.tile([128, 3 * XS_B], U8)
            xbtall = singles.tile([128, NSHOT_SLOTS, 2, C], BF16)
            xbtr = xbt.rearrange("s p (h c) -> p s h c", h=2)
            x8q_sb = singles.tile([128, NQ_SLOTS, KO, F], FP8)
            x8qr = x8q.rearrange("s p (ko f) -> p s ko f", ko=KO)
            with tc.high_priority(offset=1000):
                nc.sync.dma_start(pk0_sb, pk0[:])
                nc.sync.dma_start(pk1_sb, pk1[:])
                nc.sync.dma_start(xbtall[:, 0:2], xbtr[:, 0:2])
                nc.sync.dma_start(xbtall[:, 2:4], xbtr[:, 2:4])
                nc.sync.dma_start(x8q_sb[:, 0:5], x8qr[:, 0:5])

            wt_sb = pk0_sb[:, 0:WT_B].bitcast(FP8).rearrange(
                "p (ko o) -> p ko o", ko=KO
            )
            x8_sh = [
                pk0_sb[:, WT_B : WT_B + XS_B]
                .bitcast(FP8)
                .rearrange("p (ko f) -> p ko f", ko=KO)
            ] + [
                pk1_sb[:, i * XS_B : (i + 1) * XS_B]
                .bitcast(FP8)
                .rearrange("p (ko f) -> p ko f", ko=KO)
                for i in range(3)
            ]
            bs_sb = pk0_sb[:, WT_B + XS_B :].bitcast(F32)
            bias_sb = bs_sb[:, 0:KO]
            sw_sb = bs_sb[:, KO:].rearrange("p (s m) -> p s m", s=NSHOT_SLOTS)

            # eb[c] = exp(b[c]); ebsw = eb-weighted one-hot class rows
            eb = singles.tile([128, KO], F32)
            nc.scalar.activation(eb, bias_sb, EXP)
            eb2_bf = singles.tile([128, KO, 1], BF16)
            nc.scalar.activation(eb2_bf[:, :, 0], bias_sb, EXP, scale=2.0)
            ebC4 = singles.tile([128, KO, 128], BF16)
            nc.vector.tensor_copy(ebC4, eb[:, :, None].to_broadcast([128, KO, 128]))
            ebsw = singles.tile([128, KO, WAY, NSHOT_SLOTS], F32)
            nc.vector.tensor_tensor(
                ebsw,
                sw_sb.rearrange("p s m -> p m s")[:, None, :, :].to_broadcast(
                    [128, KO, WAY, NSHOT_SLOTS]
                ),
                eb[:, :, None, None].to_broadcast([128, KO, WAY, NSHOT_SLOTS]),
                MULT,
            )
            out_sb = singles.tile([1, NQ_SLOTS * WAY], F32)

            # score/norm PSUM: [query, f-half, 5 scores + |q|^2] for all 10
            # queries plus the shot scratch, all in one bank.
            psall = psmall.tile([128, 512], F32)
            psT = psall[:, 0 : NQ_SLOTS * 2 * (WAY + 1)].rearrange(
                "p (a h m) -> p a h m", a=NQ_SLOTS, h=2
            )
            ps_s4 = psall[:, 136 : 136 + NSHOT_SLOTS]
            ps_sn = psall[:, 144 : 144 + WAY]
            red_ps = psall[:, 160 : 160 + NSHOT_SLOTS * KO]
            sacc_ps = psall[:, 176 : 176 + NSHOT_SLOTS * KO]

            # ---------------- PE warmup (bridges the pstate ramp) ----------
            for w in range(N_WARM):
                ps_w = pscratch.tile([128, 256], F32, tag="warm", name=f"warm{w}")
                nc.tensor.matmul(ps_w, warm_bf[:, :128], warm_bf, start=True, stop=True)

            xm_tiles = [None] * NQ_SLOTS
            xm2_tiles = [None] * NQ_SLOTS

            # ---------------- shot slots (transposed conv: [f, (h,c)]) -----
            # per-slot: conv -> exp (split h halves) -> mask-mul -> column
            # matmuls -> per-slot denominator/contrib accumulated into proto.
            rS = work.tile([128, NSHOT_SLOTS], F32, tag="rS")
            proto_acc = work.tile([128, KO, WAY, NSHOT_SLOTS], F32, tag="pacc")
            for i in range(NSHOT_SLOTS):
                ps = pconv.tile([128, 2, C], F32, tag="conv", name=f"convT{i}")
                with tc.high_priority():
                    for h in range(2):
                        for k2 in range(KO // 2):
                            nc.tensor.matmul(
                                ps[:, h, :],
                                x8_sh[i][
                                    :, 2 * k2 : 2 * k2 + 2, 128 * h : 128 * (h + 1)
                                ],
                                wt_sb[:, 2 * k2 : 2 * k2 + 2, :],
                                start=(k2 == 0),
                                stop=(k2 == KO // 2 - 1),
                                perf_mode=DR,
                            )
                xeT = xepool.tile([128, 2, C], BF16, tag="xe")
                with tc.high_priority():
                    nc.scalar.activation(xeT, ps, EXP, scale=1.0 / W_SCALE)
                xmT = xms_pool.tile([128, 2, C], BF16, tag="xms")
                with tc.high_priority():
                    nc.vector.tensor_mul(xmT, xbtall[:, i], xeT)
                # per-channel masked sums / exp-sums as tiny col matmuls;
                # each PSUM accumulation pair (h0 start, h1 stop) stays
                # adjacent -- interleaving other matmuls inside an open
                # accumulation group corrupts it on hardware.
                with tc.high_priority():
                    for k in range(KO):
                        for h in range(2):
                            nc.tensor.matmul(
                                sacc_ps[:, KO * i + k : KO * i + k + 1],
                                xeT[:, h, 128 * k : 128 * (k + 1)],
                                onesC_bf[:, 0:1],
                                start=(h == 0),
                                stop=(h == 1),
                            )
                    for k in range(KO):
                        for h in range(2):
                            nc.tensor.matmul(
                                red_ps[:, KO * i + k : KO * i + k + 1],
                                xmT[:, h, 128 * k : 128 * (k + 1)],
                                onesC_bf[:, 0:1],
                                start=(h == 0),
                                stop=(h == 1),
                            )

            # ---------------- shot epilogue (batched, v1 form) -------------
            saccS = work.tile([128, NSHOT_SLOTS * KO], BF16, tag="saccS")
            with tc.high_priority():
                nc.vector.tensor_copy(saccS, sacc_ps)
                sv = saccS.rearrange("p (i k) -> p i k", i=NSHOT_SLOTS)
                for k in range(KO):
                    nc.tensor.matmul(
                        ps_s4,
                        ebC4[:, k, :],
                        sv[:, :, k],
                        start=(k == 0),
                        stop=(k == KO - 1),
                    )
                contrib1 = work.tile([128, KO, WAY, NSHOT_SLOTS], F32, tag="c1")
                nc.vector.tensor_tensor(
                    contrib1,
                    red_ps.rearrange("p (i k) -> p i k", i=NSHOT_SLOTS)
                    .rearrange("p i k -> p k i")[:, :, None, :]
                    .to_broadcast([128, KO, WAY, NSHOT_SLOTS]),
                    ebsw,
                    MULT,
                )
                nc.vector.reciprocal(rS, ps_s4)
                nc.vector.tensor_tensor(
                    proto_acc,
                    contrib1,
                    rS[:, None, None, :].to_broadcast([128, KO, WAY, NSHOT_SLOTS]),
                    MULT,
                )

            with tc.high_priority():
                proto = work.tile([128, KO, WAY, 1], BF16, tag="proto")
                with nc.allow_low_precision(reason="bf16 prototype exchange"):
                    red_inst = nc.vector.reduce_sum(
                        proto, proto_acc, axis=mybir.AxisListType.X
                    )
                ar_in = dram.tile([128, KO, WAY], BF16, tag="ar_in")
                ar_out = dram.tile([N_CORES, 128, KO, WAY], BF16, tag="ar_out")
                nc.sync.dma_start(ar_in, proto[:, :, :, 0])

            nc.gpsimd.collective_compute(
                "AllGather",
                mybir.AluOpType.bypass,
                replica_groups=[list(range(N_CORES))],
                ins=[ar_in[:].opt()],
                outs=[ar_out[:].opt()],
            )

            # remaining query inputs: SP slots behind ar_in's, so their
            # transfers never queue ahead of the collective input
            xbq_sb = singles.tile([128, NQ_SLOTS, KO, F], BF16)
            xbqr = xbq.rearrange("s p (ko f) -> p s ko f", ko=KO)
            with tc.high_priority(offset=-40000):
                nc.sync.dma_start(x8q_sb[:, 5:10], x8qr[:, 5:10])
                for j in range(0, NQ_SLOTS, 2):
                    nc.sync.dma_start(xbq_sb[:, j : j + 2], xbqr[:, j : j + 2])

            # ---------------- query slots ----------------
            def norm_mms(j):
                xm2 = xm2_tiles[j]
                for h in range(2):
                    for k in range(KO):
                        nc.tensor.matmul(
                            psT[:, j, h, WAY : WAY + 1],
                            xm2[:, k, 128 * h : 128 * (h + 1)],
                            eb2_bf[:, k, :],
                            start=(k == 0),
                            stop=(k == KO - 1),
                        )

            for j in range(NQ_SLOTS):
                ps = pconv.tile([128, KO, F], F32, tag="conv", name=f"conv{j}")
                for oo in range(KO):
                    for k2 in range(KO // 2):
                        nc.tensor.matmul(
                            ps[:, oo, :],
                            wt_sb[:, 2 * k2 : 2 * k2 + 2, 128 * oo : 128 * (oo + 1)],
                            x8q_sb[:, j, 2 * k2 : 2 * k2 + 2, :],
                            start=(k2 == 0),
                            stop=(k2 == KO // 2 - 1),
                            perf_mode=DR,
                        )
                xe = xepool.tile([128, KO, F], BF16, tag="xe")
                nc.scalar.activation(xe, ps, EXP, scale=1.0 / W_SCALE)
                xm = xmq_pool.tile([128, KO, F], BF16, tag="xmq")
                xm_tiles[j] = xm
                nc.vector.tensor_mul(xm, xbq_sb[:, j], xe)
                xm2 = xm2_pool.tile([128, KO, F], BF16, tag="xm2")
                xm2_tiles[j] = xm2
                nc.vector.tensor_mul(xm2, xm, xm)
                if j > 0:
                    norm_mms(j - 1)
            norm_mms(NQ_SLOTS - 1)

            # per-position 10/|q| for all queries (collective-independent)
            lnq = work.tile([128, NQ_SLOTS, 2], F32, tag="lnq")
            nc.scalar.activation(lnq, psT[:, :, :, WAY], LN, bias=c_eps)
            rq = work.tile([128, NQ_SLOTS, 2], F32, tag="rq")
            nc.scalar.activation(rq, lnq, EXP, bias=c_ln10, scale=-0.5)

            # ---------------- consume AllGather ----------------
            if True:
                protoAll = work.tile([128, N_CORES, KO, WAY], BF16, tag="protoAll")
                nc.sync.dma_start(protoAll, ar_out.rearrange("r p k m -> p r k m"))
                protoG = work.tile([128, KO, WAY, 1], F32, tag="protoG")
                nc.vector.reduce_sum(
                    protoG,
                    protoAll.rearrange("p r k m -> p k m r"),
                    axis=mybir.AxisListType.X,
                )
                s_hat = work.tile([128, KO, WAY], BF16, tag="s_hat")
                nc.vector.tensor_tensor(
                    s_hat,
                    protoG[:, :, :, 0],
                    eb[:, :, None].to_broadcast([128, KO, WAY]),
                    MULT,
                )
                # ---- parallel branch: rqrsn = rq[f] * 10/||protoG||[m] ----
                protosq = work.tile([128, KO, WAY], F32, tag="protosq")
                nc.vector.tensor_mul(protosq, protoG[:, :, :, 0], protoG[:, :, :, 0])
                for k in range(KO):
                    nc.tensor.matmul(
                        ps_sn,
                        onesC_f32,
                        protosq[:, k, :],
                        start=(k == 0),
                        stop=(k == KO - 1),
                    )
                snln = work.tile([128, WAY], F32, tag="snln")
                nc.scalar.activation(snln, ps_sn, LN, bias=c_eps)
                rsnb = work.tile([128, WAY], F32, tag="rsnb")
                nc.scalar.activation(rsnb, snln, EXP, scale=-0.5)

                # ---- scores (transposed matmuls) ----
                for j in range(NQ_SLOTS):
                    xm = xm_tiles[j]
                    for h in range(2):
                        for k in range(KO):
                            nc.tensor.matmul(
                                psT[:, j, h, 0:WAY],
                                xm[:, k, 128 * h : 128 * (h + 1)],
                                s_hat[:, k, :],
                                start=(k == 0),
                                stop=(k == KO - 1),
                            )
                rqrsn = work.tile([128, NQ_SLOTS, 2, WAY], F32, tag="rqrsn")
                nc.vector.tensor_tensor(
                    rqrsn,
                    rq[:, :, :, None].to_broadcast([128, NQ_SLOTS, 2, WAY]),
                    rsnb[:, None, None, :].to_broadcast([128, NQ_SLOTS, 2, WAY]),
                    MULT,
                )
                # ---------------- batched class softmax + spatial mean -----
                L = work.tile([128, NQ_SLOTS, 2, WAY], BF16, tag="L")
                nc.vector.tensor_tensor(L, psT[:, :, :, 0:WAY], rqrsn, MULT)
                E = work.tile([128, NQ_SLOTS, 2, WAY], BF16, tag="E")
                nc.scalar.activation(E, L, EXP)
                D = work.tile([128, NQ_SLOTS, 2, 1], F32, tag="D")
                nc.vector.reduce_sum(D, E, axis=mybir.AxisListType.X)
                R = work.tile([128, NQ_SLOTS, 2, 1], BF16, tag="R")
                with nc.allow_low_precision(reason="bf16 softmax denominators"):
                    nc.vector.reciprocal(R, D)
                psO = pscratch.tile([128, 256], F32, tag="warm", name="psO")
                for a in range(NQ_SLOTS):
                    for h in range(2):
                        nc.tensor.matmul(
                            psO[:1, a * WAY : (a + 1) * WAY],
                            R[:, a, h, :],
                            E[:, a, h, :],
                            start=(h == 0),
                            stop=(h == 1),
                        )
                nc.vector.tensor_scalar_mul(
                    out_sb, psO[:1, : NQ_SLOTS * WAY], 1.0 / F
                )
                nc.sync.dma_start(out[:], out_sb[0:1, :])

    nc.finalize()
    return nc


_NC_CACHE = {}


def _get_nc():
    if "nc" not in _NC_CACHE:
        _NC_CACHE["nc"] = build_nc()
    return _NC_CACHE["nc"]


def _assignments():
    """Per-core (shot global ids, query global ids)."""
    shots = [20 * c + j for c in range(WAY) for j in range(SHOT)]
    queries = [20 * c + SHOT + j for c in range(WAY) for j in range(15)]
    so = np.cumsum([0] + SHOTS_PER_CORE)
    qo = np.cumsum([0] + QUERIES_PER_CORE)
    return [
        (shots[so[k] : so[k + 1]], queries[qo[k] : qo[k + 1]]) for k in range(N_CORES)
    ]


def _pack_slots(x_np, dtype):
    """[n, C, F] -> [n, 128, KO*F] p-major layout (1KB+ contiguous runs)."""
    n = x_np.shape[0]
    v = x_np.reshape(n, KO, 128, F).transpose(0, 2, 1, 3).reshape(n, 128, KO * F)
    return np.ascontiguousarray(v).astype(dtype)


def _pack_wt(W, dtype, scale=1.0):
    w = np.ascontiguousarray(W.T * scale)
    return np.ascontiguousarray(
        w.reshape(KO, 128, C).transpose(1, 0, 2).reshape(128, KO * C)
    ).astype(dtype)


def _make_in_maps(x, W, b):
    assign = _assignments()
    wt8 = _pack_wt(W, ml_dtypes.float8_e4m3, W_SCALE)  # [128, KO*C]
    in_maps = []
    for k in range(N_CORES):
        s_list, q_list = assign[k]
        xs_core = np.zeros((NSLOTS, C, F), dtype=np.float32)
        xs_core[: len(s_list)] = x[s_list]
        xs_core[NSHOT_SLOTS : NSHOT_SLOTS + len(q_list)] = x[q_list]
        sw_core = np.zeros((NSHOT_SLOTS, WAY), dtype=np.float32)
        for slot, g in enumerate(s_list):
            sw_core[slot, g // 20] = 1.0
        bias_p = np.ascontiguousarray(b.reshape(KO, 128).T)  # [128, KO]
        bsw = np.concatenate(
            [
                bias_p,
                np.broadcast_to(
                    sw_core.reshape(1, NSHOT_SLOTS * WAY), (128, NSHOT_SLOTS * WAY)
                ),
            ],
            axis=1,
        ).astype(np.float32)
        x8 = _pack_slots(xs_core, ml_dtypes.float8_e4m3)  # [14, 128, KO*F]
        pk0 = np.concatenate(
            [
                wt8.view(np.uint8),
                x8[0].view(np.uint8),
                np.ascontiguousarray(bsw).view(np.uint8),
            ],
            axis=1,
        )
        pk1 = np.concatenate(
            [x8[1].view(np.uint8), x8[2].view(np.uint8), x8[3].view(np.uint8)],
            axis=1,
        )
        # transposed shot slots: xbt[i, p, h*C + c] = x[i, c, h*128 + p]
        xbt_np = np.ascontiguousarray(
            xs_core[:NSHOT_SLOTS]
            .reshape(NSHOT_SLOTS, C, 2, 128)
            .transpose(0, 3, 2, 1)
            .reshape(NSHOT_SLOTS, 128, 2 * C)
        ).astype(ml_dtypes.bfloat16)
        m = {
            "pk0": np.ascontiguousarray(pk0),
            "pk1": np.ascontiguousarray(pk1),
            "xbt": xbt_np,
            "x8q": x8[NSHOT_SLOTS:],
            "xbq": _pack_slots(xs_core[NSHOT_SLOTS:], ml_dtypes.bfloat16),
        }
        in_maps.append(m)
    return in_maps


def kernel(x, W, b):
    x = np.asarray(x, dtype=np.float32).reshape(100, C, F)
    W = np.asarray(W, dtype=np.float32)
    b = np.asarray(b, dtype=np.float32)

    nc = _get_nc()
    in_maps = _make_in_maps(x, W, b)
    res = run_bass_kernel_spmd(nc, in_maps, core_ids=list(range(N_CORES)))

    assign = _assignments()
    final = np.zeros((75, WAY), dtype=np.float32)
    for k in range(N_CORES):
        out_core = np.asarray(res.results[k]["out"], dtype=np.float32).reshape(
            NQ_SLOTS, WAY
        )
        _, q_list = assign[k]
        for slot, g in enumerate(q_list):
            c, j = divmod(g, 20)
            final[15 * c + (j - SHOT)] = out_core[slot]
    return final
